# revision 1
# baseline (speedup 1.0000x reference)
"""Trainium2 Bass kernel for DeformableQuantizer (vq_codebook).

Forward value of the reference:
    cb = BASE_CODEBOOK + deform_scale * delta            # [8, 3]
    per 3-elem group z: L_k = (2 z.cb_k - |cb_k|^2)/T    # softmax logits
    out = sum_k softmax(L)_k * cb_k

Device pipeline, per 96-feature chunk (32 groups) and 512-token supertile,
with x host-pretransposed to feature-major (plus a ones row carrying the
-|cb_k|^2/T - C bias; C=50 keeps exp() in fp32 range, softmax-invariant):

    stage 1 (PE):  L[g*4+k, tok] = W1^T @ xT        (bf16 hi/lo 3-term split:
                                                     W1hi@xhi + W1hi@xlo + W1lo@xhi)
    exp (ACT):     E = exp(L)  (psum -> sbuf, rounded to fp32r)
    stage 2 (PE):  [num | den] = W2^T @ E  (fp32r; 96 numerator + 32 denom rows)

Host divides num/den and re-transposes. Sharding: pure data parallel, one
batch element (4096 tokens) per NeuronCore.
"""

import itertools

import numpy as np

GROUP_DIM = 3
TEMP = 0.3
C_SHIFT = 50.0

N_CORES = 8
B, S, D = 8, 4096, 768
S_TILE = 512                # tokens per supertile
N_SUPER = S // S_TILE
N_CHUNKS = 8                # 96-feature chunks per 768 features
CHUNK_F = 96
CHUNK_G = 32

STAGE1 = "hilo"             # "hilo" (bf16 3-term) | "f32r" | "f32"
STAGE2 = "float32r"         # mybir dtype name for stage-2

_BASE_CODEBOOK = np.asarray(
    list(itertools.product([-1.0, 1.0], repeat=GROUP_DIM)), dtype=np.float32
)

_CACHE: dict = {}


def _build_bass(stage1: str = STAGE1, stage2: str = STAGE2):
    import concourse.bacc as bacc
    import concourse.tile as tile
    from concourse import mybir

    s2_dt = getattr(mybir.dt, stage2)
    s1_dt = mybir.dt.bfloat16 if stage1 == "hilo" else getattr(
        mybir.dt, {"f32r": "float32r", "f32": "float32"}[stage1]
    )
    f32 = mybir.dt.float32
    FREE = N_CHUNKS * S_TILE

    nc = bacc.Bacc()
    if stage1 == "hilo":
        xhi = nc.declare_dram_parameter("xhi", [N_SUPER, 97, FREE], s1_dt, False)
        xlo = nc.declare_dram_parameter("xlo", [N_SUPER, 97, FREE], s1_dt, False)
        w1hi = nc.declare_dram_parameter("w1hi", [97, 256], s1_dt, False)
        w1lo = nc.declare_dram_parameter("w1lo", [97, 256], s1_dt, False)
    else:
        xhi = nc.declare_dram_parameter("xhi", [N_SUPER, 97, FREE], s1_dt, False)
        w1hi = nc.declare_dram_parameter("w1hi", [97, 256], s1_dt, False)
    w2 = nc.declare_dram_parameter("w2", [128, 256], s2_dt, False)
    x3 = nc.declare_dram_parameter("x3", [32, 96], s2_dt, False)
    out = nc.declare_dram_parameter("out", [N_SUPER, 96, FREE], f32, True)

    with tile.TileContext(nc) as tc:
        with (
            tc.tile_pool(name="wpool", bufs=1) as wpool,
            tc.tile_pool(name="xpool", bufs=2) as xpool,
            tc.tile_pool(name="epool", bufs=3) as epool,
            tc.tile_pool(name="opool", bufs=2) as opool,
            tc.tile_pool(name="p1pool", bufs=2, space="PSUM") as p1pool,
            tc.tile_pool(name="p2pool", bufs=2, space="PSUM") as p2pool,
            tc.tile_pool(name="p3pool", bufs=2, space="PSUM") as p3pool,
        ):
            w1hi_sb = wpool.tile([97, 256], s1_dt, name="w1hi_sb")
            nc.sync.dma_start(out=w1hi_sb, in_=w1hi[:])
            if stage1 == "hilo":
                w1lo_sb = wpool.tile([97, 256], s1_dt, name="w1lo_sb")
                nc.sync.dma_start(out=w1lo_sb, in_=w1lo[:])
            w2_sb = wpool.tile([128, 256], s2_dt, name="w2_sb")
            nc.sync.dma_start(out=w2_sb, in_=w2[:])
            x3_sb = wpool.tile([128, 96], s2_dt, name="x3_sb")
            nc.sync.dma_start(out=x3_sb[96:128, :], in_=x3[:])

            for t in range(N_SUPER):
                xhi_sb = xpool.tile([97, FREE], s1_dt, name="xhi_sb")
                nc.sync.dma_start(out=xhi_sb, in_=xhi[t])
                if stage1 == "hilo":
                    xlo_sb = xpool.tile([97, FREE], s1_dt, name="xlo_sb")
                    nc.sync.dma_start(out=xlo_sb, in_=xlo[t])
                out_sb = opool.tile([96, FREE], f32, name="out_sb")
                for c in range(N_CHUNKS):
                    sl = slice(S_TILE * c, S_TILE * (c + 1))
                    ps1 = p1pool.tile([128, 2 * S_TILE], f32, name="ps1")
                    for half in range(2):
                        wsl = slice(128 * half, 128 * (half + 1))
                        oseg = ps1[:, S_TILE * half : S_TILE * (half + 1)]
                        if stage1 == "hilo":
                            nc.tensor.matmul(
                                oseg, w1hi_sb[:, wsl], xhi_sb[:, sl],
                                start=True, stop=False)
                            nc.tensor.matmul(
                                oseg, w1hi_sb[:, wsl], xlo_sb[:, sl],
                                start=False, stop=False)
                            nc.tensor.matmul(
                                oseg, w1lo_sb[:, wsl], xhi_sb[:, sl],
                                start=False, stop=True)
                        else:
                            nc.tensor.matmul(
                                oseg, w1hi_sb[:, wsl], xhi_sb[:, sl],
                                start=True, stop=True)
                    e_sb = epool.tile([128, 2 * S_TILE], s2_dt, name="e_sb")
                    nc.scalar.activation(
                        e_sb, ps1, mybir.ActivationFunctionType.Exp)
                    ps2 = p2pool.tile([128, S_TILE], f32, name="ps2")
                    nc.tensor.matmul(
                        ps2, w2_sb[:, 0:128], e_sb[:, 0:S_TILE],
                        start=True, stop=False)
                    nc.tensor.matmul(
                        ps2, w2_sb[:, 128:256], e_sb[:, S_TILE : 2 * S_TILE],
                        start=False, stop=True)
                    # on-device division: recip(den rows 96:128, lane-aligned),
                    # expand [32]->[96] via constant matmul, multiply num rows
                    r_sb = epool.tile([128, S_TILE], s2_dt, name="r_sb")
                    with nc.allow_low_precision(reason="fp32r recip for PE"):
                        nc.vector.reciprocal(r_sb[96:128, :], ps2[96:128, :])
                    rx = p3pool.tile([96, S_TILE], f32, name="rx")
                    nc.tensor.matmul(
                        rx, x3_sb[96:128, :], r_sb[96:128, :],
                        start=True, stop=True, tile_position=(96, 0))
                    # PSUM has one DVE read port: stage rx in SBUF (ACT copy)
                    rx_sb = epool.tile([96, S_TILE], f32, name="rx_sb")
                    nc.scalar.copy(rx_sb, rx)
                    nc.vector.tensor_mul(
                        out_sb[:, sl], ps2[0:96, :], rx_sb)
                nc.sync.dma_start(out=out[t], in_=out_sb)
    nc.compile()
    return nc


def _weights(delta: np.ndarray, deform_scale: np.ndarray):
    cb = (_BASE_CODEBOOK + np.float32(deform_scale) * delta.astype(np.float32)).astype(
        np.float32
    )
    cbn = (cb * cb).sum(1)

    w1 = np.zeros((97, 256), np.float32)   # [feat+ones, (lo|hi 128) = g*4+k]
    w2 = np.zeros((128, 256), np.float32)  # [g*4+k, (lo|hi 128) = 96 num + 32 den]
    for half in range(2):
        for g in range(CHUNK_G):
            for k in range(4):
                kk = 4 * half + k
                m = 128 * half + g * 4 + k
                for c in range(GROUP_DIM):
                    w1[3 * g + c, m] = 2.0 * cb[kk, c] / TEMP
                    w2[g * 4 + k, 128 * half + 3 * g + c] = cb[kk, c]
                w1[96, m] = -cbn[kk] / TEMP - C_SHIFT
                w2[g * 4 + k, 128 * half + 96 + g] = 1.0
    return w1, w2


def _bf16_split(a: np.ndarray):
    import ml_dtypes

    hi = a.astype(ml_dtypes.bfloat16)
    lo = (a - hi.astype(np.float32)).astype(ml_dtypes.bfloat16)
    return hi, lo


def _prep_core(x_core: np.ndarray) -> np.ndarray:
    # [4096, 768] -> [N_SUPER, 97, N_CHUNKS*S_TILE]; free = chunk*S_TILE + tok
    xr = x_core.reshape(N_SUPER, S_TILE, N_CHUNKS, CHUNK_F)  # t, u, c, f
    xprep = np.ones((N_SUPER, 97, N_CHUNKS, S_TILE), np.float32)
    xprep[:, :CHUNK_F] = xr.transpose(0, 3, 2, 1)
    return np.ascontiguousarray(xprep.reshape(N_SUPER, 97, N_CHUNKS * S_TILE))


def _x3() -> np.ndarray:
    x3 = np.zeros((32, 96), np.float32)
    for g in range(CHUNK_G):
        x3[g, 3 * g : 3 * g + 3] = 1.0
    return x3


def _postprocess(outs: list[np.ndarray]) -> np.ndarray:
    ys = []
    for o in outs:
        y = o.reshape(N_SUPER, CHUNK_G, 3, N_CHUNKS, S_TILE)
        ys.append(y.transpose(0, 4, 3, 1, 2).reshape(S, D))
    return np.stack(ys).astype(np.float32)


def make_in_maps(x, delta, deform_scale, stage1: str = STAGE1):
    w1, w2 = _weights(delta, deform_scale)
    maps = []
    if stage1 == "hilo":
        w1hi, w1lo = _bf16_split(w1)
        for b in range(N_CORES):
            xp = _prep_core(x[b])
            xh, xl = _bf16_split(xp)
            maps.append({"xhi": xh, "xlo": xl, "w1hi": w1hi, "w1lo": w1lo,
                         "w2": w2, "x3": _x3()})
    else:
        for b in range(N_CORES):
            maps.append({"xhi": _prep_core(x[b]), "w1hi": w1, "w2": w2,
                         "x3": _x3()})
    return maps


def kernel(x, delta, deform_scale):
    from concourse.bass_utils import run_bass_kernel_spmd

    x = np.asarray(x, dtype=np.float32)
    delta = np.asarray(delta, dtype=np.float32)
    deform_scale = np.asarray(deform_scale, dtype=np.float32)

    if "nc" not in _CACHE:
        _CACHE["nc"] = _build_bass()
    nc = _CACHE["nc"]

    in_maps = make_in_maps(x, delta, deform_scale)
    res = run_bass_kernel_spmd(nc, in_maps, core_ids=list(range(N_CORES)))
    return _postprocess([r["out"] for r in res.results])


if __name__ == "__main__":
    x = np.random.randn(B, S, D).astype(np.float32)
    delta = (np.random.randn(8, 3) * 0.1).astype(np.float32)
    ds = np.float32(0.05)
    y = kernel(x, delta, ds)
    print("out", y.shape, y.dtype)



# revision 3
# speedup vs baseline: 5.5289x; 5.5289x over previous
"""Trainium2 Bass kernel for DeformableQuantizer (vq_codebook).

Forward value of the reference:
    cb = BASE_CODEBOOK + deform_scale * delta            # [8, 3]
    per 3-elem group z: L_k = 2 z.cb_k / T               # (affine part of logits)
    E'_k = exp(L_k - C2);  B_k = exp(-|cb_k|^2/T - C3)   # C2+C3 = softmax shift
    out_c = (sum_k E'_k B_k cb_kc) / (sum_k E'_k B_k)    # softmax-weighted combine

Device pipeline, per 96-feature chunk (32 groups) and 512-token supertile,
with x host-pretransposed to feature-major fp16 (96 partitions - a multiple
of 8 so the HW DMA descriptor spread across the 16 SDMA engines engages):

    stage 1 (PE):  L[4g+k, tok] = W1^T @ xT     (fp16, 2 matmuls: code-halves)
    exp (ACT):     E = exp(L - 30)              (scalar bias; per-code bias is
                                                 folded multiplicatively into W2)
    stage 2 (PE):  [num96 | den32] = W2^T @ E   (fp32r, 2 matmuls)
    copy (DVE):    psum fp32 -> sbuf bf16
    out DMA:       [128, 4096] bf16 per supertile

Host divides num/den and re-transposes. Sharding: pure data parallel, one
batch element (4096 tokens) per NeuronCore.
"""

import itertools

import numpy as np

GROUP_DIM = 3
TEMP = 0.3
C2 = 30.0                   # scalar shift inside exp
C3 = 20.0                   # shift folded into W2 (total softmax shift 50)

N_CORES = 8
B, S, D = 8, 4096, 768
S_TILE = 512                # tokens per supertile
N_SUPER = S // S_TILE
N_CHUNKS = 8                # 96-feature chunks per 768 features
CHUNK_F = 96
CHUNK_G = 32

_BASE_CODEBOOK = np.asarray(
    list(itertools.product([-1.0, 1.0], repeat=GROUP_DIM)), dtype=np.float32
)

_CACHE: dict = {}


def _build_bass():
    import concourse.bacc as bacc
    import concourse.tile as tile
    from concourse import mybir

    f32 = mybir.dt.float32
    f32r = mybir.dt.float32r
    f16 = mybir.dt.float16
    bf16 = mybir.dt.bfloat16
    FREE = N_CHUNKS * S_TILE

    nc = bacc.Bacc()
    xin = nc.declare_dram_parameter("xin", [N_SUPER, CHUNK_F, FREE], f16, False)
    w1 = nc.declare_dram_parameter("w1", [CHUNK_F, 256], f16, False)
    w2 = nc.declare_dram_parameter("w2", [128, 256], f32r, False)
    out = nc.declare_dram_parameter("out", [N_SUPER, 128, FREE], bf16, True)

    # exp's float bias needs a pre-registered const AP (Bass.__init__ only
    # registers 0.0/1.0); mirror its register_const_ap here.
    bias_t = nc.alloc_sbuf_tensor(f"const-float32-{-C2}", [128, 1], f32)
    nc.gpsimd.memset(bias_t.ap(), -C2)
    nc.const_aps.aps[(f32, -C2)] = bias_t.ap()
    nc.all_engine_barrier()

    with tile.TileContext(nc) as tc:
        with (
            tc.tile_pool(name="wpool", bufs=1) as wpool,
            tc.tile_pool(name="xpool", bufs=2) as xpool,
            tc.tile_pool(name="epool", bufs=3) as epool,
            tc.tile_pool(name="opool", bufs=2) as opool,
            tc.tile_pool(name="p1pool", bufs=2, space="PSUM") as p1pool,
            tc.tile_pool(name="p2pool", bufs=4, space="PSUM") as p2pool,
        ):
            w1_sb = wpool.tile([CHUNK_F, 256], f16, name="w1_sb")
            nc.sync.dma_start(out=w1_sb, in_=w1[:])
            w2_sb = wpool.tile([128, 256], f32r, name="w2_sb")
            nc.sync.dma_start(out=w2_sb, in_=w2[:])

            for t in range(N_SUPER):
                x_sb = xpool.tile([CHUNK_F, FREE], f16, name="x_sb")
                nc.sync.dma_start(out=x_sb, in_=xin[t])
                o_sb = opool.tile([128, FREE], bf16, name="o_sb")
                for c in range(N_CHUNKS):
                    sl = slice(S_TILE * c, S_TILE * (c + 1))
                    ps1 = p1pool.tile([128, 2 * S_TILE], f32, name="ps1")
                    nc.tensor.matmul(
                        ps1[:, 0:S_TILE], w1_sb[:, 0:128], x_sb[:, sl],
                        start=True, stop=True)
                    nc.tensor.matmul(
                        ps1[:, S_TILE : 2 * S_TILE], w1_sb[:, 128:256],
                        x_sb[:, sl], start=True, stop=True)
                    e_sb = epool.tile([128, 2 * S_TILE], f32r, name="e_sb")
                    nc.scalar.activation(
                        e_sb, ps1, mybir.ActivationFunctionType.Exp, bias=-C2)
                    ps2 = p2pool.tile([128, S_TILE], f32, name="ps2")
                    nc.tensor.matmul(
                        ps2, w2_sb[:, 0:128], e_sb[:, 0:S_TILE],
                        start=True, stop=False)
                    nc.tensor.matmul(
                        ps2, w2_sb[:, 128:256], e_sb[:, S_TILE : 2 * S_TILE],
                        start=False, stop=True)
                    nc.vector.tensor_copy(o_sb[:, sl], ps2)
                nc.sync.dma_start(out=out[t], in_=o_sb)
    nc.compile()
    return nc


def _weights(delta: np.ndarray, deform_scale: np.ndarray):
    cb = (_BASE_CODEBOOK + np.float32(deform_scale) * delta.astype(np.float32))
    cbn = (cb * cb).sum(1)
    bk = np.exp(-cbn / TEMP - C3).astype(np.float32)     # per-code folded bias

    w1 = np.zeros((CHUNK_F, 256), np.float32)  # [feat, (half 128) = 4g+k]
    w2 = np.zeros((128, 256), np.float32)      # [4g+k, (half 128) = 96num+32den]
    for half in range(2):
        for g in range(CHUNK_G):
            for k in range(4):
                kk = 4 * half + k
                m = 128 * half + 4 * g + k
                for c in range(GROUP_DIM):
                    w1[3 * g + c, m] = 2.0 * cb[kk, c] / TEMP
                    w2[4 * g + k, 128 * half + 3 * g + c] = cb[kk, c] * bk[kk]
                w2[4 * g + k, 128 * half + 96 + g] = bk[kk]
    return w1.astype(np.float16), w2


def _prep_core(x_core: np.ndarray) -> np.ndarray:
    # [4096, 768] -> [N_SUPER, 96, N_CHUNKS*S_TILE]; free = chunk*S_TILE + tok
    xr = x_core.reshape(N_SUPER, S_TILE, N_CHUNKS, CHUNK_F)  # t, u, c, f
    xp = np.ascontiguousarray(xr.transpose(0, 3, 2, 1)).astype(np.float16)
    return xp.reshape(N_SUPER, CHUNK_F, N_CHUNKS * S_TILE)


def _postprocess(outs: list[np.ndarray]) -> np.ndarray:
    ys = []
    for o in outs:
        o = np.asarray(o, dtype=np.float32).reshape(
            N_SUPER, 128, N_CHUNKS, S_TILE)
        num = o[:, :96].reshape(N_SUPER, CHUNK_G, 3, N_CHUNKS, S_TILE)
        den = o[:, 96:128]                       # [t, g, c, u]
        den = np.where(den == 0.0, 1.0, den)
        q = num / den[:, :, None]
        ys.append(q.transpose(0, 4, 3, 1, 2).reshape(S, D))
    return np.stack(ys).astype(np.float32)


def make_in_maps(x, delta, deform_scale):
    w1, w2 = _weights(delta, deform_scale)
    return [
        {"xin": _prep_core(x[b]), "w1": w1, "w2": w2} for b in range(N_CORES)
    ]


def kernel(x, delta, deform_scale):
    from concourse.bass_utils import run_bass_kernel_spmd

    x = np.asarray(x, dtype=np.float32)
    delta = np.asarray(delta, dtype=np.float32)
    deform_scale = np.asarray(deform_scale, dtype=np.float32)

    if "nc" not in _CACHE:
        _CACHE["nc"] = _build_bass()
    nc = _CACHE["nc"]

    in_maps = make_in_maps(x, delta, deform_scale)
    res = run_bass_kernel_spmd(nc, in_maps, core_ids=list(range(N_CORES)))
    return _postprocess([r["out"] for r in res.results])


if __name__ == "__main__":
    x = np.random.randn(B, S, D).astype(np.float32)
    delta = (np.random.randn(8, 3) * 0.1).astype(np.float32)
    ds = np.float32(0.05)
    y = kernel(x, delta, ds)
    print("out", y.shape, y.dtype)


# revision 10
# speedup vs baseline: 6.2274x; 1.1263x over previous
"""Trainium2 Bass kernel for DeformableQuantizer (vq_codebook).

Forward value of the reference:
    cb = BASE_CODEBOOK + deform_scale * delta            # [8, 3]
    per 3-elem group z: L_k = 2 z.cb_k / T               # (affine part of logits)
    E'_k = exp(L_k - C2);  B_k = exp(-|cb_k|^2/T - C3)   # C2+C3 = softmax shift
    out_c = (sum_k E'_k B_k cb_kc) / (sum_k E'_k B_k)    # softmax-weighted combine

Device pipeline, per 96-feature chunk (32 groups) and 512-token supertile,
with x host-pretransposed to feature-major fp16 (96 partitions - a multiple
of 8 so the HW DMA descriptor spread across the 16 SDMA engines engages):

    stage 1 (PE):  L[4g+k, tok] = W1^T @ xT     (fp16, 2 matmuls: code-halves)
    exp (ACT):     E = exp(L - 30)              (scalar bias; per-code bias is
                                                 folded multiplicatively into W2)
    stage 2 (PE):  [num96 | den32] = W2^T @ E   (fp32r, 2 matmuls)
    copy (DVE):    psum fp32 -> sbuf bf16
    out DMA:       [128, 4096] bf16 per supertile

Host divides num/den and re-transposes. Sharding: pure data parallel, one
batch element (4096 tokens) per NeuronCore.
"""

import itertools

import numpy as np

GROUP_DIM = 3
TEMP = 0.3
C2 = 30.0                   # scalar shift inside exp
C3 = 20.0                   # shift folded into W2 (total softmax shift 50)

N_CORES = 8
B, S, D = 8, 4096, 768
S_TILE = 512                # tokens per supertile
N_SUPER = S // S_TILE
N_CHUNKS = 8                # 96-feature chunks per 768 features
CHUNK_F = 96
CHUNK_G = 32

_BASE_CODEBOOK = np.asarray(
    list(itertools.product([-1.0, 1.0], repeat=GROUP_DIM)), dtype=np.float32
)

_CACHE: dict = {}


def _build_bass():
    import concourse.bacc as bacc
    import concourse.tile as tile
    from concourse import mybir

    f32 = mybir.dt.float32
    f32r = mybir.dt.float32r
    f16 = mybir.dt.float16
    bf16 = mybir.dt.bfloat16
    FREE = N_CHUNKS * S_TILE

    nc = bacc.Bacc()
    xin = nc.declare_dram_parameter("xin", [N_SUPER, CHUNK_F, FREE], f16, False)
    w1 = nc.declare_dram_parameter("w1", [CHUNK_F, 256], f16, False)
    w2 = nc.declare_dram_parameter("w2", [128, 256], f32r, False)
    out = nc.declare_dram_parameter("out", [N_SUPER, 128, FREE], bf16, True)

    # exp's float bias needs a pre-registered const AP (Bass.__init__ only
    # registers 0.0/1.0); mirror its register_const_ap here.
    bias_t = nc.alloc_sbuf_tensor(f"const-float32-{-C2}", [128, 1], f32)
    nc.gpsimd.memset(bias_t.ap(), -C2)
    nc.const_aps.aps[(f32, -C2)] = bias_t.ap()
    nc.all_engine_barrier()

    with tile.TileContext(nc) as tc:
        with (
            tc.tile_pool(name="wpool", bufs=1) as wpool,
            tc.tile_pool(name="xpool", bufs=2) as xpool,
            tc.tile_pool(name="epool", bufs=3) as epool,
            tc.tile_pool(name="opool", bufs=2) as opool,
            tc.tile_pool(name="p1pool", bufs=1, space="PSUM") as p1pool,
            tc.tile_pool(name="p2pool", bufs=2, space="PSUM") as p2pool,
        ):
            w1_sb = wpool.tile([CHUNK_F, 256], f16, name="w1_sb")
            nc.sync.dma_start(out=w1_sb, in_=w1[:])
            w2_sb = wpool.tile([128, 256], f32r, name="w2_sb")
            nc.sync.dma_start(out=w2_sb, in_=w2[:])

            P = 2 * S_TILE          # 1024-token pair span

            def stage2(pend):
                ea, eb, sl, o_sb = pend
                # weight-half major so consecutive MMs share lhsT
                ps2 = p2pool.tile([128, P], f32, name="ps2")
                nc.tensor.matmul(
                    ps2[:, 0:S_TILE], w2_sb[:, 0:128], ea[:, 0:S_TILE],
                    start=True, stop=False)
                nc.tensor.matmul(
                    ps2[:, S_TILE:P], w2_sb[:, 0:128], ea[:, S_TILE:P],
                    start=True, stop=False)
                nc.tensor.matmul(
                    ps2[:, 0:S_TILE], w2_sb[:, 128:256], eb[:, 0:S_TILE],
                    start=False, stop=True)
                nc.tensor.matmul(
                    ps2[:, S_TILE:P], w2_sb[:, 128:256], eb[:, S_TILE:P],
                    start=False, stop=True)
                nc.vector.tensor_copy(o_sb[:, sl], ps2)

            # software-pipelined: pair p's stage 1 + exp is emitted before
            # pair p-1's stage 2, so the PE never waits on the newest exp.
            pend = None
            dma_pend = None
            for t in range(N_SUPER):
                x_sb = xpool.tile([CHUNK_F, FREE], f16, name="x_sb")
                nc.sync.dma_start(out=x_sb, in_=xin[t])
                o_sb = opool.tile([128, FREE], bf16, name="o_sb")
                for p in range(N_CHUNKS // 2):
                    sl = slice(P * p, P * (p + 1))
                    # stage 1: per code-half, 2 N=512 matmuls sharing lhsT
                    # (PSUM bank limits a matmul to 512 fp32 output columns)
                    lo, hi = P * p, P * p + S_TILE
                    ps1a = p1pool.tile([128, P], f32, name="ps1a")
                    nc.tensor.matmul(
                        ps1a[:, 0:S_TILE], w1_sb[:, 0:128],
                        x_sb[:, lo : lo + S_TILE], start=True, stop=True)
                    nc.tensor.matmul(
                        ps1a[:, S_TILE:P], w1_sb[:, 0:128],
                        x_sb[:, hi : hi + S_TILE], start=True, stop=True)
                    ea = epool.tile([128, P], f32r, name="ea")
                    nc.scalar.activation(
                        ea, ps1a, mybir.ActivationFunctionType.Exp, bias=-C2)
                    ps1b = p1pool.tile([128, P], f32, name="ps1b")
                    nc.tensor.matmul(
                        ps1b[:, 0:S_TILE], w1_sb[:, 128:256],
                        x_sb[:, lo : lo + S_TILE], start=True, stop=True)
                    nc.tensor.matmul(
                        ps1b[:, S_TILE:P], w1_sb[:, 128:256],
                        x_sb[:, hi : hi + S_TILE], start=True, stop=True)
                    eb = epool.tile([128, P], f32r, name="eb")
                    nc.scalar.activation(
                        eb, ps1b, mybir.ActivationFunctionType.Exp, bias=-C2)
                    if pend is not None:
                        stage2(pend)
                        if dma_pend is not None:
                            dma_t, dma_sb = dma_pend
                            nc.sync.dma_start(out=out[dma_t], in_=dma_sb)
                            dma_pend = None
                    pend = (ea, eb, sl, o_sb)
                dma_pend = (t, o_sb)
            stage2(pend)
            nc.sync.dma_start(out=out[N_SUPER - 1], in_=o_sb)
    nc.compile()
    return nc


def _weights(delta: np.ndarray, deform_scale: np.ndarray):
    cb = (_BASE_CODEBOOK + np.float32(deform_scale) * delta.astype(np.float32))
    cbn = (cb * cb).sum(1)
    bk = np.exp(-cbn / TEMP - C3).astype(np.float32)     # per-code folded bias

    w1 = np.zeros((CHUNK_F, 256), np.float32)  # [feat, (half 128) = 4g+k]
    w2 = np.zeros((128, 256), np.float32)      # [4g+k, (half 128) = 96num+32den]
    for half in range(2):
        for g in range(CHUNK_G):
            for k in range(4):
                kk = 4 * half + k
                m = 128 * half + 4 * g + k
                for c in range(GROUP_DIM):
                    w1[3 * g + c, m] = 2.0 * cb[kk, c] / TEMP
                    w2[4 * g + k, 128 * half + 3 * g + c] = cb[kk, c] * bk[kk]
                w2[4 * g + k, 128 * half + 96 + g] = bk[kk]
    return w1.astype(np.float16), w2


def _prep_core(x_core: np.ndarray) -> np.ndarray:
    # [4096, 768] -> [N_SUPER, 96, N_CHUNKS*S_TILE]; free = chunk*S_TILE + tok
    xr = x_core.reshape(N_SUPER, S_TILE, N_CHUNKS, CHUNK_F)  # t, u, c, f
    xp = np.ascontiguousarray(xr.transpose(0, 3, 2, 1)).astype(np.float16)
    return xp.reshape(N_SUPER, CHUNK_F, N_CHUNKS * S_TILE)


def _postprocess(outs: list[np.ndarray]) -> np.ndarray:
    ys = []
    for o in outs:
        o = np.asarray(o, dtype=np.float32).reshape(
            N_SUPER, 128, N_CHUNKS, S_TILE)
        num = o[:, :96].reshape(N_SUPER, CHUNK_G, 3, N_CHUNKS, S_TILE)
        den = o[:, 96:128]                       # [t, g, c, u]
        den = np.where(den == 0.0, 1.0, den)
        q = num / den[:, :, None]
        ys.append(q.transpose(0, 4, 3, 1, 2).reshape(S, D))
    return np.stack(ys).astype(np.float32)


def make_in_maps(x, delta, deform_scale):
    w1, w2 = _weights(delta, deform_scale)
    return [
        {"xin": _prep_core(x[b]), "w1": w1, "w2": w2} for b in range(N_CORES)
    ]


def kernel(x, delta, deform_scale):
    from concourse.bass_utils import run_bass_kernel_spmd

    x = np.asarray(x, dtype=np.float32)
    delta = np.asarray(delta, dtype=np.float32)
    deform_scale = np.asarray(deform_scale, dtype=np.float32)

    if "nc" not in _CACHE:
        _CACHE["nc"] = _build_bass()
    nc = _CACHE["nc"]

    in_maps = make_in_maps(x, delta, deform_scale)
    res = run_bass_kernel_spmd(nc, in_maps, core_ids=list(range(N_CORES)))
    return _postprocess([r["out"] for r in res.results])


if __name__ == "__main__":
    x = np.random.randn(B, S, D).astype(np.float32)
    delta = (np.random.randn(8, 3) * 0.1).astype(np.float32)
    ds = np.float32(0.05)
    y = kernel(x, delta, ds)
    print("out", y.shape, y.dtype)


# revision 12
# speedup vs baseline: 6.4712x; 1.0392x over previous
"""Trainium2 Bass kernel for DeformableQuantizer (vq_codebook).

Forward value of the reference:
    cb = BASE_CODEBOOK + deform_scale * delta            # [8, 3]
    per 3-elem group z: L_k = 2 z.cb_k / T               # (affine part of logits)
    E'_k = exp(L_k - C2);  B_k = exp(-|cb_k|^2/T - C3)   # C2+C3 = softmax shift
    out_c = (sum_k E'_k B_k cb_kc) / (sum_k E'_k B_k)    # softmax-weighted combine

Device pipeline, per 96-feature chunk (32 groups) and 512-token supertile,
with x host-pretransposed to feature-major fp16 (96 partitions - a multiple
of 8 so the HW DMA descriptor spread across the 16 SDMA engines engages):

    stage 1 (PE):  L[4g+k, tok] = W1^T @ xT     (fp16, 2 matmuls: code-halves)
    exp (ACT):     E = exp(L - 30)              (scalar bias; per-code bias is
                                                 folded multiplicatively into W2)
    stage 2 (PE):  [num96 | den32] = W2^T @ E   (fp32r, 2 matmuls)
    copy (DVE):    psum fp32 -> sbuf bf16
    out DMA:       [128, 4096] bf16 per supertile

Host divides num/den and re-transposes. Sharding: pure data parallel, one
batch element (4096 tokens) per NeuronCore.
"""

import itertools

import numpy as np

GROUP_DIM = 3
TEMP = 0.3
C2 = 30.0                   # scalar shift inside exp
C3 = 20.0                   # shift folded into W2 (total softmax shift 50)

N_CORES = 8
B, S, D = 8, 4096, 768
S_TILE = 512                # tokens per supertile
N_SUPER = S // S_TILE
N_CHUNKS = 8                # 96-feature chunks per 768 features
CHUNK_F = 96
CHUNK_G = 32

_BASE_CODEBOOK = np.asarray(
    list(itertools.product([-1.0, 1.0], repeat=GROUP_DIM)), dtype=np.float32
)

_CACHE: dict = {}


def _build_bass():
    import concourse.bacc as bacc
    import concourse.tile as tile
    from concourse import mybir

    f32 = mybir.dt.float32
    f32r = mybir.dt.float32r
    f16 = mybir.dt.float16
    bf16 = mybir.dt.bfloat16
    FREE = N_CHUNKS * S_TILE

    nc = bacc.Bacc()
    xin = nc.declare_dram_parameter("xin", [N_SUPER, CHUNK_F, FREE], f16, False)
    w1 = nc.declare_dram_parameter("w1", [CHUNK_F, 256], f16, False)
    w2 = nc.declare_dram_parameter("w2", [128, 256], f32r, False)
    out = nc.declare_dram_parameter("out", [N_SUPER, 128, FREE], bf16, True)

    # exp's float bias needs a pre-registered const AP (Bass.__init__ only
    # registers 0.0/1.0); mirror its register_const_ap here.
    bias_t = nc.alloc_sbuf_tensor(f"const-float32-{-C2}", [128, 1], f32)
    nc.gpsimd.memset(bias_t.ap(), -C2)
    nc.const_aps.aps[(f32, -C2)] = bias_t.ap()
    nc.all_engine_barrier()

    with tile.TileContext(nc) as tc:
        with (
            tc.tile_pool(name="wpool", bufs=1) as wpool,
            tc.tile_pool(name="xpool", bufs=8) as xpool,
            tc.tile_pool(name="epool", bufs=3) as epool,
            tc.tile_pool(name="opool", bufs=3) as opool,
            tc.tile_pool(name="p1pool", bufs=1, space="PSUM") as p1pool,
            tc.tile_pool(name="p2pool", bufs=2, space="PSUM") as p2pool,
        ):
            w1_sb = wpool.tile([CHUNK_F, 256], f16, name="w1_sb")
            nc.sync.dma_start(out=w1_sb, in_=w1[:])
            w2_sb = wpool.tile([128, 256], f32r, name="w2_sb")
            nc.sync.dma_start(out=w2_sb, in_=w2[:])

            P = 2 * S_TILE          # 1024-token pair span

            def stage2(pend):
                ea, eb, t, sl = pend
                # weight-half major so consecutive MMs share lhsT
                ps2 = p2pool.tile([128, P], f32, name="ps2")
                nc.tensor.matmul(
                    ps2[:, 0:S_TILE], w2_sb[:, 0:128], ea[:, 0:S_TILE],
                    start=True, stop=False)
                nc.tensor.matmul(
                    ps2[:, S_TILE:P], w2_sb[:, 0:128], ea[:, S_TILE:P],
                    start=True, stop=False)
                nc.tensor.matmul(
                    ps2[:, 0:S_TILE], w2_sb[:, 128:256], eb[:, 0:S_TILE],
                    start=False, stop=True)
                nc.tensor.matmul(
                    ps2[:, S_TILE:P], w2_sb[:, 128:256], eb[:, S_TILE:P],
                    start=False, stop=True)
                o_p = opool.tile([128, P], bf16, name="o_p")
                nc.vector.tensor_copy(o_p, ps2)
                nc.sync.dma_start(out=out[t][:, sl], in_=o_p)

            # software-pipelined: pair p's stage 1 + exp is emitted before
            # pair p-1's stage 2, so the PE never waits on the newest exp.
            # Pair-granular in/out DMAs shrink the pipeline head and tail.
            pend = None
            for t in range(N_SUPER):
                for p in range(N_CHUNKS // 2):
                    sl = slice(P * p, P * (p + 1))
                    x_p = xpool.tile([CHUNK_F, P], f16, name="x_p")
                    nc.sync.dma_start(out=x_p, in_=xin[t][:, sl])
                    # stage 1: per code-half, 2 N=512 matmuls sharing lhsT
                    # (PSUM bank limits a matmul to 512 fp32 output columns)
                    ps1a = p1pool.tile([128, P], f32, name="ps1a")
                    nc.tensor.matmul(
                        ps1a[:, 0:S_TILE], w1_sb[:, 0:128],
                        x_p[:, 0:S_TILE], start=True, stop=True)
                    nc.tensor.matmul(
                        ps1a[:, S_TILE:P], w1_sb[:, 0:128],
                        x_p[:, S_TILE:P], start=True, stop=True)
                    ea = epool.tile([128, P], f32r, name="ea")
                    nc.scalar.activation(
                        ea, ps1a, mybir.ActivationFunctionType.Exp, bias=-C2)
                    ps1b = p1pool.tile([128, P], f32, name="ps1b")
                    nc.tensor.matmul(
                        ps1b[:, 0:S_TILE], w1_sb[:, 128:256],
                        x_p[:, 0:S_TILE], start=True, stop=True)
                    nc.tensor.matmul(
                        ps1b[:, S_TILE:P], w1_sb[:, 128:256],
                        x_p[:, S_TILE:P], start=True, stop=True)
                    eb = epool.tile([128, P], f32r, name="eb")
                    nc.scalar.activation(
                        eb, ps1b, mybir.ActivationFunctionType.Exp, bias=-C2)
                    if pend is not None:
                        stage2(pend)
                    pend = (ea, eb, t, sl)
            stage2(pend)
    nc.compile()
    return nc


def _weights(delta: np.ndarray, deform_scale: np.ndarray):
    cb = (_BASE_CODEBOOK + np.float32(deform_scale) * delta.astype(np.float32))
    cbn = (cb * cb).sum(1)
    bk = np.exp(-cbn / TEMP - C3).astype(np.float32)     # per-code folded bias

    w1 = np.zeros((CHUNK_F, 256), np.float32)  # [feat, (half 128) = 4g+k]
    w2 = np.zeros((128, 256), np.float32)      # [4g+k, (half 128) = 96num+32den]
    for half in range(2):
        for g in range(CHUNK_G):
            for k in range(4):
                kk = 4 * half + k
                m = 128 * half + 4 * g + k
                for c in range(GROUP_DIM):
                    w1[3 * g + c, m] = 2.0 * cb[kk, c] / TEMP
                    w2[4 * g + k, 128 * half + 3 * g + c] = cb[kk, c] * bk[kk]
                w2[4 * g + k, 128 * half + 96 + g] = bk[kk]
    return w1.astype(np.float16), w2


def _prep_core(x_core: np.ndarray) -> np.ndarray:
    # [4096, 768] -> [N_SUPER, 96, N_CHUNKS*S_TILE]; free = chunk*S_TILE + tok
    xr = x_core.reshape(N_SUPER, S_TILE, N_CHUNKS, CHUNK_F)  # t, u, c, f
    xp = np.ascontiguousarray(xr.transpose(0, 3, 2, 1)).astype(np.float16)
    return xp.reshape(N_SUPER, CHUNK_F, N_CHUNKS * S_TILE)


def _postprocess(outs: list[np.ndarray]) -> np.ndarray:
    ys = []
    for o in outs:
        o = np.asarray(o, dtype=np.float32).reshape(
            N_SUPER, 128, N_CHUNKS, S_TILE)
        num = o[:, :96].reshape(N_SUPER, CHUNK_G, 3, N_CHUNKS, S_TILE)
        den = o[:, 96:128]                       # [t, g, c, u]
        den = np.where(den == 0.0, 1.0, den)
        q = num / den[:, :, None]
        ys.append(q.transpose(0, 4, 3, 1, 2).reshape(S, D))
    return np.stack(ys).astype(np.float32)


def make_in_maps(x, delta, deform_scale):
    w1, w2 = _weights(delta, deform_scale)
    return [
        {"xin": _prep_core(x[b]), "w1": w1, "w2": w2} for b in range(N_CORES)
    ]


def kernel(x, delta, deform_scale):
    from concourse.bass_utils import run_bass_kernel_spmd

    x = np.asarray(x, dtype=np.float32)
    delta = np.asarray(delta, dtype=np.float32)
    deform_scale = np.asarray(deform_scale, dtype=np.float32)

    if "nc" not in _CACHE:
        _CACHE["nc"] = _build_bass()
    nc = _CACHE["nc"]

    in_maps = make_in_maps(x, delta, deform_scale)
    res = run_bass_kernel_spmd(nc, in_maps, core_ids=list(range(N_CORES)))
    return _postprocess([r["out"] for r in res.results])


if __name__ == "__main__":
    x = np.random.randn(B, S, D).astype(np.float32)
    delta = (np.random.randn(8, 3) * 0.1).astype(np.float32)
    ds = np.float32(0.05)
    y = kernel(x, delta, ds)
    print("out", y.shape, y.dtype)


# revision 17
# speedup vs baseline: 6.6010x; 1.0201x over previous
"""Trainium2 Bass kernel for DeformableQuantizer (vq_codebook).

Forward value of the reference:
    cb = BASE_CODEBOOK + deform_scale * delta            # [8, 3]
    per 3-elem group z: L_k = 2 z.cb_k / T               # (affine part of logits)
    E'_k = exp(L_k - C2);  B_k = exp(-|cb_k|^2/T - C3)   # C2+C3 = softmax shift
    out_c = (sum_k E'_k B_k cb_kc) / (sum_k E'_k B_k)    # softmax-weighted combine

Device pipeline, per 96-feature chunk (32 groups) and 512-token supertile,
with x host-pretransposed to feature-major fp16 (96 partitions - a multiple
of 8 so the HW DMA descriptor spread across the 16 SDMA engines engages):

    stage 1 (PE):  L[4g+k, tok] = W1^T @ xT     (fp16, 2 matmuls: code-halves)
    exp (ACT):     E = exp(L - 30)              (scalar bias; per-code bias is
                                                 folded multiplicatively into W2)
    stage 2 (PE):  [num96 | den32] = W2^T @ E   (fp32r, 2 matmuls)
    copy (DVE):    psum fp32 -> sbuf bf16
    out DMA:       [128, 4096] bf16 per supertile

Host divides num/den and re-transposes. Sharding: pure data parallel, one
batch element (4096 tokens) per NeuronCore.
"""

import itertools

import numpy as np

GROUP_DIM = 3
TEMP = 0.3
C2 = 30.0                   # scalar shift inside exp
C3 = 20.0                   # shift folded into W2 (total softmax shift 50)

N_CORES = 8
B, S, D = 8, 4096, 768
S_TILE = 512                # tokens per supertile
N_SUPER = S // S_TILE
N_CHUNKS = 8                # 96-feature chunks per 768 features
CHUNK_F = 96
CHUNK_G = 32

_BASE_CODEBOOK = np.asarray(
    list(itertools.product([-1.0, 1.0], repeat=GROUP_DIM)), dtype=np.float32
)

_CACHE: dict = {}


def _build_bass():
    import concourse.bacc as bacc
    import concourse.tile as tile
    from concourse import mybir

    f32 = mybir.dt.float32
    f32r = mybir.dt.float32r
    f16 = mybir.dt.float16
    bf16 = mybir.dt.bfloat16
    FREE = N_CHUNKS * S_TILE

    nc = bacc.Bacc()
    xin = nc.declare_dram_parameter("xin", [N_SUPER, CHUNK_F, FREE], f16, False)
    w1 = nc.declare_dram_parameter("w1", [CHUNK_F, 256], f16, False)
    w2 = nc.declare_dram_parameter("w2", [128, 256], f32r, False)
    bias = nc.declare_dram_parameter("bias", [128, 1], f32, False)
    out = nc.declare_dram_parameter("out", [N_SUPER, 128, FREE], bf16, True)

    with tile.TileContext(nc) as tc:
        with (
            tc.tile_pool(name="wpool", bufs=1) as wpool,
            tc.tile_pool(name="xpool", bufs=8) as xpool,
            tc.tile_pool(name="epool", bufs=3) as epool,
            tc.tile_pool(name="opool", bufs=3) as opool,
            tc.tile_pool(name="p1pool", bufs=1, space="PSUM") as p1pool,
            tc.tile_pool(name="p2pool", bufs=2, space="PSUM") as p2pool,
        ):
            P = 2 * S_TILE          # 1024-token pair span

            # first pair's x load goes out before the (DMA-issue-serialized)
            # weight loads so compute can start as early as possible
            x_first = xpool.tile([CHUNK_F, P], f16, name="x_p")
            nc.sync.dma_start(out=x_first, in_=xin[0][:, 0:P])
            w1_sb = wpool.tile([CHUNK_F, 256], f16, name="w1_sb")
            nc.sync.dma_start(out=w1_sb, in_=w1[:])
            w2_sb = wpool.tile([128, 256], f32r, name="w2_sb")
            nc.sync.dma_start(out=w2_sb, in_=w2[:])
            bias_sb = wpool.tile([128, 1], f32, name="bias_sb")
            nc.sync.dma_start(out=bias_sb, in_=bias[:])

            def stage2(pend):
                ea, eb, t, sl = pend
                # weight-half major so consecutive MMs share lhsT
                ps2 = p2pool.tile([128, P], f32, name="ps2")
                nc.tensor.matmul(
                    ps2[:, 0:S_TILE], w2_sb[:, 0:128], ea[:, 0:S_TILE],
                    start=True, stop=False)
                nc.tensor.matmul(
                    ps2[:, S_TILE:P], w2_sb[:, 0:128], ea[:, S_TILE:P],
                    start=True, stop=False)
                nc.tensor.matmul(
                    ps2[:, 0:S_TILE], w2_sb[:, 128:256], eb[:, 0:S_TILE],
                    start=False, stop=True)
                nc.tensor.matmul(
                    ps2[:, S_TILE:P], w2_sb[:, 128:256], eb[:, S_TILE:P],
                    start=False, stop=True)
                o_p = opool.tile([128, P], bf16, name="o_p")
                nc.vector.tensor_copy(o_p, ps2)
                nc.sync.dma_start(out=out[t][:, sl], in_=o_p)

            # software-pipelined: pair p's stage 1 + exp is emitted before
            # pair p-1's stage 2, so the PE never waits on the newest exp.
            # Pair-granular in/out DMAs shrink the pipeline head and tail.
            pend = None
            for t in range(N_SUPER):
                for p in range(N_CHUNKS // 2):
                    sl = slice(P * p, P * (p + 1))
                    if t == 0 and p == 0:
                        x_p = x_first
                    else:
                        x_p = xpool.tile([CHUNK_F, P], f16, name="x_p")
                        nc.sync.dma_start(out=x_p, in_=xin[t][:, sl])
                    # stage 1: per code-half, 2 N=512 matmuls sharing lhsT
                    # (PSUM bank limits a matmul to 512 fp32 output columns)
                    ps1a = p1pool.tile([128, P], f32, name="ps1a")
                    nc.tensor.matmul(
                        ps1a[:, 0:S_TILE], w1_sb[:, 0:128],
                        x_p[:, 0:S_TILE], start=True, stop=True)
                    nc.tensor.matmul(
                        ps1a[:, S_TILE:P], w1_sb[:, 0:128],
                        x_p[:, S_TILE:P], start=True, stop=True)
                    ea = epool.tile([128, P], f32r, name="ea")
                    nc.scalar.activation(
                        ea, ps1a, mybir.ActivationFunctionType.Exp,
                        bias=bias_sb)
                    ps1b = p1pool.tile([128, P], f32, name="ps1b")
                    nc.tensor.matmul(
                        ps1b[:, 0:S_TILE], w1_sb[:, 128:256],
                        x_p[:, 0:S_TILE], start=True, stop=True)
                    nc.tensor.matmul(
                        ps1b[:, S_TILE:P], w1_sb[:, 128:256],
                        x_p[:, S_TILE:P], start=True, stop=True)
                    eb = epool.tile([128, P], f32r, name="eb")
                    nc.scalar.activation(
                        eb, ps1b, mybir.ActivationFunctionType.Exp,
                        bias=bias_sb)
                    if pend is not None:
                        stage2(pend)
                    pend = (ea, eb, t, sl)
            stage2(pend)
    nc.compile()
    return nc


def _weights(delta: np.ndarray, deform_scale: np.ndarray):
    cb = (_BASE_CODEBOOK + np.float32(deform_scale) * delta.astype(np.float32))
    cbn = (cb * cb).sum(1)
    bk = np.exp(-cbn / TEMP - C3).astype(np.float32)     # per-code folded bias

    w1 = np.zeros((CHUNK_F, 256), np.float32)  # [feat, (half 128) = 4g+k]
    w2 = np.zeros((128, 256), np.float32)      # [4g+k, (half 128) = 96num+32den]
    for half in range(2):
        for g in range(CHUNK_G):
            for k in range(4):
                kk = 4 * half + k
                m = 128 * half + 4 * g + k
                for c in range(GROUP_DIM):
                    w1[3 * g + c, m] = 2.0 * cb[kk, c] / TEMP
                    w2[4 * g + k, 128 * half + 3 * g + c] = cb[kk, c] * bk[kk]
                w2[4 * g + k, 128 * half + 96 + g] = bk[kk]
    return w1.astype(np.float16), w2


def _prep_core(x_core: np.ndarray) -> np.ndarray:
    # [4096, 768] -> [N_SUPER, 96, N_CHUNKS*S_TILE]; free = chunk*S_TILE + tok
    xr = x_core.reshape(N_SUPER, S_TILE, N_CHUNKS, CHUNK_F)  # t, u, c, f
    xp = np.ascontiguousarray(xr.transpose(0, 3, 2, 1)).astype(np.float16)
    return xp.reshape(N_SUPER, CHUNK_F, N_CHUNKS * S_TILE)


def _postprocess(outs: list[np.ndarray]) -> np.ndarray:
    ys = []
    for o in outs:
        o = np.asarray(o, dtype=np.float32).reshape(
            N_SUPER, 128, N_CHUNKS, S_TILE)
        num = o[:, :96].reshape(N_SUPER, CHUNK_G, 3, N_CHUNKS, S_TILE)
        den = o[:, 96:128]                       # [t, g, c, u]
        den = np.where(den == 0.0, 1.0, den)
        q = num / den[:, :, None]
        ys.append(q.transpose(0, 4, 3, 1, 2).reshape(S, D))
    return np.stack(ys).astype(np.float32)


def make_in_maps(x, delta, deform_scale):
    w1, w2 = _weights(delta, deform_scale)
    bias = np.full((128, 1), -C2, np.float32)
    return [
        {"xin": _prep_core(x[b]), "w1": w1, "w2": w2, "bias": bias}
        for b in range(N_CORES)
    ]


def kernel(x, delta, deform_scale):
    from concourse.bass_utils import run_bass_kernel_spmd

    x = np.asarray(x, dtype=np.float32)
    delta = np.asarray(delta, dtype=np.float32)
    deform_scale = np.asarray(deform_scale, dtype=np.float32)

    if "nc" not in _CACHE:
        _CACHE["nc"] = _build_bass()
    nc = _CACHE["nc"]

    in_maps = make_in_maps(x, delta, deform_scale)
    res = run_bass_kernel_spmd(nc, in_maps, core_ids=list(range(N_CORES)))
    return _postprocess([r["out"] for r in res.results])


if __name__ == "__main__":
    x = np.random.randn(B, S, D).astype(np.float32)
    delta = (np.random.randn(8, 3) * 0.1).astype(np.float32)
    ds = np.float32(0.05)
    y = kernel(x, delta, ds)
    print("out", y.shape, y.dtype)
